# revision 36
# baseline (speedup 1.0000x reference)
"""Trainium2 Bass kernel for additive-attention scoring:

    out[b, m, n] = sum_h v[h] * tanh(queries[b, m, h] + keys[b, n, h])

Shapes: queries (4, 1024, 128) f32, keys (4, 1024, 128) f32, v (128,) f32
Output: (4, 1024, 1024) f32.

Sharding: 8 cores; core c handles batch c//2, m-half c%2 (512 m rows each).

Algorithm: instead of materializing the 536M-element tanh (ScalarE-bound
at ~437us), factor the bivariate kernel through its functional SVD:

    tanh(q + k) ~= sum_r a_r(q) * b_r(k),   r < R  (R = 8)

where a_r/b_r are the singular functions of the integral operator under
the N(0,1) x N(0,1) product measure (the actual q,k distribution; R=8
weighted tail ~1.2e-3).  The contraction becomes a pure TensorE matmul
with contraction dim R*128:

    out[m, n] = sum_r sum_h [s_h a_r(q_mh)] * [t_h b_r(k_nh)]

with v split as s_h = sign(v_h) sqrt|v_h|, t_h = sqrt|v_h| so both factor
planes stay in fp8's normal range.  The host evaluates the singular
functions by linear interpolation on a fine grid and uploads one fp8e4m3
feature plane per core.  All matmuls run in DoubleRow mode (two 128-deep
fp8 contraction chunks per pass, 0.5 cycles/row):

  - ranks 0+1 (97% of the mass) are hi/lo split: G = Gh + Gl, K = Kh + Kl
    (each fp8), and three DoubleRow passes compute Gh*Kh + Gl*Kh + Gh*Kl
    per rank -- ~2^-8 effective precision, the lo*lo term is negligible;
  - ranks 2..7 ride as three plain DoubleRow pairs.

Measured end-to-end rel err 5.6e-3 against the 2e-2 gate.  The device
kernel is rank-major accumulation into all 8 PSUM banks (4 m-tiles x 2
n-halves), chunked input DMA pipelined against the matmuls, staggered
eviction on the last pair (PSUM->SBUF f16 copies spread over DVE, ACT
and GPSIMD; per-tile output DMAs), plus a dependency-free warm-up matmul
burst so the PE p-state ramp (1.2 -> 2.4 GHz after ~3us of continuous
busy) completes before the real matmuls arrive.

Known toolchain quirk: walrus accepts at most one sync-wait per
instruction, so after Tile scheduling, _sanitize_waits drops redundant
same-engine waits and hoists the rest onto single-wait NoOps.
"""

import os
import numpy as np

from concourse import bass, mybir
from concourse.tile import TileContext
from concourse.bass_utils import run_bass_kernel_spmd

B, M, N, H = 4, 1024, 1024, 128
NCORES = 8
MPC = (B * M) // NCORES  # 512 m-rows per core

R = 8  # SVD rank
GRID = 1408
LO, HI = -5.46, 5.46

F32 = mybir.dt.float32
F16 = mybir.dt.float16
FP8 = mybir.dt.float8e4

# feat8 column layout (fp8, per partition-row):
#   SecA [0,1024):      Ghi pair, tile-interleaved: per t: [Gh0[t]|Gh1[t]]
#   SecB [1024,3072):   Khi pair, per half: [Kh0[hh]|Kh1[hh]]
#   SecC [3072,4096):   Glo pair, tile-interleaved
#   SecD [4096,6144):   Klo pair, per half
#   SecE.. [6144+3072p): rank pair p: [G interleave (1024) | K (2048)]
SECA, SECB, SECC, SECD, SECE = 0, 1024, 3072, 4096, 6144
NPAIR = (R - 2) // 2
FCOLS = SECE + NPAIR * 3072

_CACHE = {}

# Filled by kernel() after each run (exec_time_ns etc) for the dev harness.
last_result = None


_ENGINE_SEM_PREFIX = {
    mybir.EngineType.Activation: "Activation_",
    mybir.EngineType.PE: "PE_",
    mybir.EngineType.DVE: "DVE_",
    mybir.EngineType.Pool: "Pool_",
    mybir.EngineType.SP: "SP_",
}


def _sanitize_waits(nc):
    """Walrus in this toolchain accepts at most ONE sync-wait per
    instruction. Drop redundant same-engine completion waits (engine FIFO
    already orders them), then hoist the rest onto dedicated single-wait
    NoOps that run just before the instruction on the same engine queue."""
    for f in nc.m.functions:
        for blk in f.blocks:
            i = 0
            while i < len(blk.instructions):
                inst = blk.instructions[i]
                si = inst.sync_info
                if si is None or len(si.on_wait) <= 1:
                    i += 1
                    continue
                waits = list(si.on_wait)
                pref = _ENGINE_SEM_PREFIX.get(inst.engine)
                if pref is not None:
                    waits = [
                        w for w in waits
                        if not (w.ant_name or "").startswith(pref)
                    ]
                for w in waits[:-1]:
                    nop = mybir.InstNoOp(
                        name=nc.get_next_instruction_name(),
                        sync_info=mybir.SyncInfo(on_wait=[w], on_update=[]),
                        bass_nofuse=True,
                        engine=inst.engine,
                    )
                    nc.register_instruction(nop)
                    blk.instructions.insert(i, nop)
                    i += 1
                si.on_wait = waits[-1:]
                inst.sync_info = si
                i += 1


def _svd_tables():
    """Singular-function tables of tanh(q+k) under the N(0,1) x N(0,1)
    product measure (with a small weight floor so the fit stays sane at
    the +-5 sigma tail points that do occur in the fixed inputs)."""
    grid = np.linspace(LO, HI, GRID)
    dx = grid[1] - grid[0]
    dens = np.exp(-grid * grid / 2.0) / np.sqrt(2.0 * np.pi)
    w = np.maximum(dens, 1e-7) * dx
    sw = np.sqrt(w)
    T = np.tanh(grid[:, None] + grid[None, :])
    U, S, Vt = np.linalg.svd(sw[:, None] * T * sw[None, :])
    A = (U[:, :R] * np.sqrt(S[:R])[None, :]) / sw[:, None]   # q-side
    Bt = (Vt[:R].T * np.sqrt(S[:R])[None, :]) / sw[:, None]  # k-side
    return grid, A, Bt


def _build_nc():
    from contextlib import ExitStack

    NWARM = int(os.environ.get("KWARM", "10"))

    nc = bass.Bass()
    feat8 = nc.declare_dram_parameter("feat8", [H, FCOLS], FP8, isOutput=False)
    out = nc.declare_dram_parameter("out", [MPC, N], F16, isOutput=True)

    ntiles = MPC // 128
    DR = mybir.MatmulPerfMode.DoubleRow

    with TileContext(nc) as tc, ExitStack() as ctx:
        const = ctx.enter_context(tc.tile_pool(name="const", bufs=1))
        opool = ctx.enter_context(tc.tile_pool(name="outp", bufs=1))
        ppool = ctx.enter_context(tc.tile_pool(name="acc", bufs=1, space="PSUM"))

        FT8 = const.tile([H, FCOLS], FP8)
        # One private staging buffer per m-tile: a shared/double-buffered
        # pool would add WAR edges (copy of tile t+2 waiting on tile t's
        # output DMA +900ns sem).
        obs = [
            opool.tile([H, N], F16, name=f"ob{t}", tag=f"ob{t}")
            for t in range(ntiles)
        ]

        # Input DMA chunks, in consumption order: Gh+Kh(hh0), Kh(hh1), Gl,
        # the plain pairs (split G+K(hh0) / K(hh1)), and the small Kl
        # section last -- its pass is consumed last, so a small final
        # chunk minimizes the last-chunk wait and the pair starvation all
        # lands at the cheap front of the PE schedule.
        # 7 chunks: more would oversubscribe HWDGE descriptor generation
        # (632ns per dma_start, serialized) and starve the DMA engines.
        ranges = [(0, 2048), (2048, SECD), (SECD, SECE)]
        for p in range(NPAIR - 1):
            pb = SECE + p * 3072
            ranges.append((pb, pb + 3072))
        pb = SECE + (NPAIR - 1) * 3072
        ranges += [(pb, pb + 2048), (pb + 2048, pb + 3072)]
        # Alternate the issuing ring: one sequencer's ~650ns per dma_start
        # can't keep 16 DMA engines fed with sub-700ns chunks.
        for i, (c0, c1) in enumerate(ranges):
            eng = nc.sync if i % 2 == 0 else nc.scalar
            eng.dma_start(FT8[:, c0:c1], feat8[:, c0:c1])

        # All 8 accumulators (4 m-tiles x 2 n-halves) live simultaneously:
        # exactly the 8 PSUM banks.
        acc = [
            [
                ppool.tile([128, 512], F32, name=f"a{t}_{h}", tag=f"a{t}_{h}")
                for h in range(2)
            ]
            for t in range(ntiles)
        ]

        # PE p-state warm-up: keep TensorE continuously busy from t~0 so
        # the ramp (full speed after ~3us of busy) completes before the
        # real matmuls.  The dummies read obs[0] (written only by the much
        # later tile-0 copy, so they issue immediately with no
        # dependencies) and their results are never read (start=True on
        # the first real matmul resets the accumulator).
        jk = obs[0][:, 0:128]
        for i in range(NWARM):
            nc.tensor.matmul(
                acc[0][0][:, 0:128], jk, jk,
                start=True, stop=True, skip_group_check=True,
            )

        def pair2(apx):
            return apx.rearrange("p (two f) -> p two f", two=2)

        def g_at(base, t):
            return pair2(FT8[:, base + t * 256: base + t * 256 + 256])

        def k_at(base, hh):
            return pair2(FT8[:, base + hh * 1024: base + (hh + 1) * 1024])

        def mm(t, h, gbase, kbase, start=False, stop=False):
            nc.tensor.matmul(
                acc[t][h][:], g_at(gbase, t), k_at(kbase, h),
                start=start, stop=stop, skip_group_check=True, perf_mode=DR,
            )

        # Pass 1 (Gh*Kh, ranks 0+1): a0 for all tiles needs only chunk 0,
        # a1 needs chunk 1.
        for t in range(ntiles):
            mm(t, 0, SECA, SECB, start=True)
        for t in range(ntiles):
            mm(t, 1, SECA, SECB, start=True)
        # Pass 2 (Gl*Kh) and pass 3 (Gh*Kl).
        for t in range(ntiles):
            mm(t, 0, SECC, SECB)
            mm(t, 1, SECC, SECB)
        for t in range(ntiles):
            mm(t, 0, SECA, SECD)
            mm(t, 1, SECA, SECD)
        # Plain rank pairs; the last one runs tile-complete (both halves
        # consecutively, paced by its split K chunk) so whole tiles finish
        # 214ns apart and the eviction pipeline (PSUM->SBUF f16 copies on
        # DVE+ACT, output DMAs on otherwise-idle rings) starts as early as
        # the input stream allows -- the final chunk is the 364ns K-half.
        cp = {
            "v": lambda d, s: nc.vector.tensor_copy(d, s),
            "s": lambda d, s: nc.scalar.copy(d, s),
        }
        assign = os.environ.get("KCOPY", "svsv,vsvs")
        a0eng, a1eng = assign.split(",")
        for p in range(NPAIR - 1):
            gb = SECE + p * 3072
            for t in range(ntiles):
                mm(t, 0, gb, gb + 1024)
                mm(t, 1, gb, gb + 1024)
        gb = SECE + (NPAIR - 1) * 3072
        for t in range(ntiles):
            mm(t, 0, gb, gb + 1024, stop=True)
            mm(t, 1, gb, gb + 1024, stop=True)
            cp[a0eng[t]](obs[t][:, 0:512], acc[t][0][:])
            cp[a1eng[t]](obs[t][:, 512:1024], acc[t][1][:])
            rows = slice(t * 128, (t + 1) * 128)
            if t < ntiles - 1:
                nc.sync.dma_start(out[rows, :], obs[t][:])
            else:
                # Split the last tile's DMA by half across two idle rings
                # so the final (critical) transfer is small and starts
                # right after its own half's copy.
                nc.sync.dma_start(out[rows, 0:512], obs[t][:, 0:512])
                nc.scalar.dma_start(out[rows, 512:1024], obs[t][:, 512:1024])
    _sanitize_waits(nc)
    return nc


def kernel(queries, keys, v):
    global last_result
    queries = np.asarray(queries, dtype=np.float32)
    keys = np.asarray(keys, dtype=np.float32)
    v = np.asarray(v, dtype=np.float32)

    import ml_dtypes

    if "nc" not in _CACHE:
        _CACHE["nc"] = _build_nc()
        _CACHE["tables"] = _svd_tables()
    nc = _CACHE["nc"]
    grid, A, Bt = _CACHE["tables"]

    F8 = ml_dtypes.float8_e4m3
    sv = np.sqrt(np.abs(v))
    gs = (np.sign(v) * sv).astype(np.float64)

    in_maps = []
    for c in range(NCORES):
        b, half = c // 2, c % 2
        qs = queries[b, half * MPC: (half + 1) * MPC, :].astype(np.float64)
        ks = keys[b].astype(np.float64)
        gf = [np.interp(qs, grid, A[:, r]).T * gs[:, None] for r in range(R)]
        kf = [np.interp(ks, grid, Bt[:, r]).T * sv[:, None] for r in range(R)]

        feat8 = np.empty((H, FCOLS), dtype=F8)

        def put_g(base, g0, g1):
            for t in range(4):
                feat8[:, base + t * 256: base + t * 256 + 128] = g0[
                    :, t * 128: (t + 1) * 128
                ].astype(F8)
                feat8[:, base + t * 256 + 128: base + (t + 1) * 256] = g1[
                    :, t * 128: (t + 1) * 128
                ].astype(F8)

        def put_k(base, k0, k1):
            for hh in range(2):
                cols = slice(hh * 512, (hh + 1) * 512)
                feat8[:, base + hh * 1024: base + hh * 1024 + 512] = k0[
                    :, cols
                ].astype(F8)
                feat8[:, base + hh * 1024 + 512: base + (hh + 1) * 1024] = k1[
                    :, cols
                ].astype(F8)

        gh = [gf[r].astype(F8).astype(np.float64) for r in range(2)]
        kh = [kf[r].astype(F8).astype(np.float64) for r in range(2)]
        put_g(SECA, gh[0], gh[1])
        put_k(SECB, kh[0], kh[1])
        put_g(SECC, gf[0] - gh[0], gf[1] - gh[1])
        put_k(SECD, kf[0] - kh[0], kf[1] - kh[1])
        for p in range(NPAIR):
            r0, r1 = 2 + 2 * p, 3 + 2 * p
            put_g(SECE + p * 3072, gf[r0], gf[r1])
            put_k(SECE + p * 3072 + 1024, kf[r0], kf[r1])
        in_maps.append({"feat8": np.ascontiguousarray(feat8)})

    trace = bool(os.environ.get("KERNEL_TRACE"))
    res = run_bass_kernel_spmd(
        nc, in_maps, core_ids=list(range(NCORES)), trace=trace
    )
    last_result = res

    full = np.empty((B, M, N), np.float32)
    for c in range(NCORES):
        b, half = c // 2, c % 2
        full[b, half * MPC: (half + 1) * MPC, :] = res.results[c]["out"].astype(
            np.float32
        )
    return full


# revision 37
# speedup vs baseline: 1.0624x; 1.0624x over previous
"""Trainium2 Bass kernel for additive-attention scoring:

    out[b, m, n] = sum_h v[h] * tanh(queries[b, m, h] + keys[b, n, h])

Shapes: queries (4, 1024, 128) f32, keys (4, 1024, 128) f32, v (128,) f32
Output: (4, 1024, 1024) f32.

Sharding: 8 cores; core c handles batch c//2, m-half c%2 (512 m rows each).

Algorithm: instead of materializing the 536M-element tanh (ScalarE-bound
at ~437us), factor the bivariate kernel through its functional SVD:

    tanh(q + k) ~= sum_r a_r(q) * b_r(k),   r < R  (R = 8)

where a_r/b_r are the singular functions of the integral operator under
the N(0,1) x N(0,1) product measure (the actual q,k distribution; R=8
weighted tail ~1.2e-3).  The contraction becomes a pure TensorE matmul
with contraction dim R*128:

    out[m, n] = sum_r sum_h [s_h a_r(q_mh)] * [t_h b_r(k_nh)]

with v split as s_h = sign(v_h) sqrt|v_h|, t_h = sqrt|v_h| so both factor
planes stay in fp8's normal range.  The host evaluates the singular
functions by linear interpolation on a fine grid and uploads one fp8e4m3
feature plane per core.  All matmuls run in DoubleRow mode (two 128-deep
fp8 contraction chunks per pass, 0.5 cycles/row):

  - ranks 0+1 (97% of the mass) are hi/lo split: G = Gh + Gl, K = Kh + Kl
    (each fp8), and three DoubleRow passes compute Gh*Kh + Gl*Kh + Gh*Kl
    per rank -- ~2^-8 effective precision, the lo*lo term is negligible;
  - ranks 2..7 ride as three plain DoubleRow pairs.

Measured end-to-end rel err 5.6e-3 against the 2e-2 gate.  The device
kernel is rank-major accumulation into all 8 PSUM banks (4 m-tiles x 2
n-halves), chunked input DMA pipelined against the matmuls, staggered
eviction on the last pair (PSUM->SBUF f16 copies spread over DVE, ACT
and GPSIMD; per-tile output DMAs), plus a dependency-free warm-up matmul
burst so the PE p-state ramp (1.2 -> 2.4 GHz after ~3us of continuous
busy) completes before the real matmuls arrive.

Known toolchain quirk: walrus accepts at most one sync-wait per
instruction, so after Tile scheduling, _sanitize_waits drops redundant
same-engine waits and hoists the rest onto single-wait NoOps.
"""

import os
import numpy as np

from concourse import bass, mybir
from concourse.tile import TileContext
from concourse.bass_utils import run_bass_kernel_spmd

B, M, N, H = 4, 1024, 1024, 128
NCORES = 8
MPC = (B * M) // NCORES  # 512 m-rows per core

R = 6  # SVD rank (tail 4.1e-3; with fp8 rounding rel err 7.3e-3 vs 2e-2 gate)
GRID = 1408
LO, HI = -5.46, 5.46

F32 = mybir.dt.float32
F16 = mybir.dt.float16
FP8 = mybir.dt.float8e4

# feat8 column layout (fp8, per partition-row):
#   SecA [0,1024):      Ghi pair, tile-interleaved: per t: [Gh0[t]|Gh1[t]]
#   SecB [1024,3072):   Khi pair, per half: [Kh0[hh]|Kh1[hh]]
#   SecC [3072,4096):   Glo pair, tile-interleaved
#   SecD [4096,6144):   Klo pair, per half
#   SecE.. [6144+3072p): rank pair p: [G interleave (1024) | K (2048)]
SECA, SECB, SECC, SECD, SECE = 0, 1024, 3072, 4096, 6144
NPAIR = (R - 2) // 2
FCOLS = SECE + NPAIR * 3072

_CACHE = {}

# Filled by kernel() after each run (exec_time_ns etc) for the dev harness.
last_result = None


_ENGINE_SEM_PREFIX = {
    mybir.EngineType.Activation: "Activation_",
    mybir.EngineType.PE: "PE_",
    mybir.EngineType.DVE: "DVE_",
    mybir.EngineType.Pool: "Pool_",
    mybir.EngineType.SP: "SP_",
}


def _sanitize_waits(nc):
    """Walrus in this toolchain accepts at most ONE sync-wait per
    instruction. Drop redundant same-engine completion waits (engine FIFO
    already orders them), then hoist the rest onto dedicated single-wait
    NoOps that run just before the instruction on the same engine queue."""
    for f in nc.m.functions:
        for blk in f.blocks:
            i = 0
            while i < len(blk.instructions):
                inst = blk.instructions[i]
                si = inst.sync_info
                if si is None or len(si.on_wait) <= 1:
                    i += 1
                    continue
                waits = list(si.on_wait)
                pref = _ENGINE_SEM_PREFIX.get(inst.engine)
                if pref is not None:
                    waits = [
                        w for w in waits
                        if not (w.ant_name or "").startswith(pref)
                    ]
                for w in waits[:-1]:
                    nop = mybir.InstNoOp(
                        name=nc.get_next_instruction_name(),
                        sync_info=mybir.SyncInfo(on_wait=[w], on_update=[]),
                        bass_nofuse=True,
                        engine=inst.engine,
                    )
                    nc.register_instruction(nop)
                    blk.instructions.insert(i, nop)
                    i += 1
                si.on_wait = waits[-1:]
                inst.sync_info = si
                i += 1


def _svd_tables():
    """Singular-function tables of tanh(q+k) under the N(0,1) x N(0,1)
    product measure (with a small weight floor so the fit stays sane at
    the +-5 sigma tail points that do occur in the fixed inputs)."""
    grid = np.linspace(LO, HI, GRID)
    dx = grid[1] - grid[0]
    dens = np.exp(-grid * grid / 2.0) / np.sqrt(2.0 * np.pi)
    w = np.maximum(dens, 1e-7) * dx
    sw = np.sqrt(w)
    T = np.tanh(grid[:, None] + grid[None, :])
    U, S, Vt = np.linalg.svd(sw[:, None] * T * sw[None, :])
    A = (U[:, :R] * np.sqrt(S[:R])[None, :]) / sw[:, None]   # q-side
    Bt = (Vt[:R].T * np.sqrt(S[:R])[None, :]) / sw[:, None]  # k-side
    return grid, A, Bt


def _build_nc():
    from contextlib import ExitStack

    NWARM = int(os.environ.get("KWARM", "10"))

    nc = bass.Bass()
    feat8 = nc.declare_dram_parameter("feat8", [H, FCOLS], FP8, isOutput=False)
    out = nc.declare_dram_parameter("out", [MPC, N], F16, isOutput=True)

    ntiles = MPC // 128
    DR = mybir.MatmulPerfMode.DoubleRow

    with TileContext(nc) as tc, ExitStack() as ctx:
        const = ctx.enter_context(tc.tile_pool(name="const", bufs=1))
        opool = ctx.enter_context(tc.tile_pool(name="outp", bufs=1))
        ppool = ctx.enter_context(tc.tile_pool(name="acc", bufs=1, space="PSUM"))

        FT8 = const.tile([H, FCOLS], FP8)
        # One private staging buffer per m-tile: a shared/double-buffered
        # pool would add WAR edges (copy of tile t+2 waiting on tile t's
        # output DMA +900ns sem).
        obs = [
            opool.tile([H, N], F16, name=f"ob{t}", tag=f"ob{t}")
            for t in range(ntiles)
        ]

        # Input DMA chunks, in consumption order: Gh+Kh(hh0), Kh(hh1), Gl,
        # the plain pairs (split G+K(hh0) / K(hh1)), and the small Kl
        # section last -- its pass is consumed last, so a small final
        # chunk minimizes the last-chunk wait and the pair starvation all
        # lands at the cheap front of the PE schedule.
        # 7 chunks: more would oversubscribe HWDGE descriptor generation
        # (632ns per dma_start, serialized) and starve the DMA engines.
        ranges = [(0, 2048), (2048, SECD), (SECD, SECE)]
        for p in range(NPAIR - 1):
            pb = SECE + p * 3072
            ranges.append((pb, pb + 3072))
        pb = SECE + (NPAIR - 1) * 3072
        ranges += [(pb, pb + 2048), (pb + 2048, pb + 3072)]
        # Alternate the issuing ring: one sequencer's ~650ns per dma_start
        # can't keep 16 DMA engines fed with sub-700ns chunks.
        for i, (c0, c1) in enumerate(ranges):
            eng = nc.sync if i % 2 == 0 else nc.scalar
            eng.dma_start(FT8[:, c0:c1], feat8[:, c0:c1])

        # All 8 accumulators (4 m-tiles x 2 n-halves) live simultaneously:
        # exactly the 8 PSUM banks.
        acc = [
            [
                ppool.tile([128, 512], F32, name=f"a{t}_{h}", tag=f"a{t}_{h}")
                for h in range(2)
            ]
            for t in range(ntiles)
        ]

        # PE p-state warm-up: keep TensorE continuously busy from t~0 so
        # the ramp (full speed after ~3us of busy) completes before the
        # real matmuls.  The dummies read obs[0] (written only by the much
        # later tile-0 copy, so they issue immediately with no
        # dependencies) and their results are never read (start=True on
        # the first real matmul resets the accumulator).
        jk = obs[0][:, 0:128]
        for i in range(NWARM):
            nc.tensor.matmul(
                acc[0][0][:, 0:128], jk, jk,
                start=True, stop=True, skip_group_check=True,
            )

        def pair2(apx):
            return apx.rearrange("p (two f) -> p two f", two=2)

        def g_at(base, t):
            return pair2(FT8[:, base + t * 256: base + t * 256 + 256])

        def k_at(base, hh):
            return pair2(FT8[:, base + hh * 1024: base + (hh + 1) * 1024])

        def mm(t, h, gbase, kbase, start=False, stop=False):
            nc.tensor.matmul(
                acc[t][h][:], g_at(gbase, t), k_at(kbase, h),
                start=start, stop=stop, skip_group_check=True, perf_mode=DR,
            )

        # Pass 1 (Gh*Kh, ranks 0+1): a0 for all tiles needs only chunk 0,
        # a1 needs chunk 1.
        for t in range(ntiles):
            mm(t, 0, SECA, SECB, start=True)
        for t in range(ntiles):
            mm(t, 1, SECA, SECB, start=True)
        # Pass 2 (Gl*Kh) and pass 3 (Gh*Kl).
        for t in range(ntiles):
            mm(t, 0, SECC, SECB)
            mm(t, 1, SECC, SECB)
        for t in range(ntiles):
            mm(t, 0, SECA, SECD)
            mm(t, 1, SECA, SECD)
        # Plain rank pairs; the last one runs tile-complete (both halves
        # consecutively, paced by its split K chunk) so whole tiles finish
        # 214ns apart and the eviction pipeline (PSUM->SBUF f16 copies on
        # DVE+ACT, output DMAs on otherwise-idle rings) starts as early as
        # the input stream allows -- the final chunk is the 364ns K-half.
        cp = {
            "v": lambda d, s: nc.vector.tensor_copy(d, s),
            "s": lambda d, s: nc.scalar.copy(d, s),
        }
        assign = os.environ.get("KCOPY", "svsv,vsvs")
        a0eng, a1eng = assign.split(",")
        for p in range(NPAIR - 1):
            gb = SECE + p * 3072
            for t in range(ntiles):
                mm(t, 0, gb, gb + 1024)
                mm(t, 1, gb, gb + 1024)
        gb = SECE + (NPAIR - 1) * 3072
        for t in range(ntiles):
            mm(t, 0, gb, gb + 1024, stop=True)
            mm(t, 1, gb, gb + 1024, stop=True)
            cp[a0eng[t]](obs[t][:, 0:512], acc[t][0][:])
            cp[a1eng[t]](obs[t][:, 512:1024], acc[t][1][:])
            rows = slice(t * 128, (t + 1) * 128)
            if t < ntiles - 1:
                nc.sync.dma_start(out[rows, :], obs[t][:])
            else:
                # Split the last tile's DMA by half across two idle rings
                # so the final (critical) transfer is small and starts
                # right after its own half's copy.
                nc.sync.dma_start(out[rows, 0:512], obs[t][:, 0:512])
                nc.scalar.dma_start(out[rows, 512:1024], obs[t][:, 512:1024])
    _sanitize_waits(nc)
    return nc


def kernel(queries, keys, v):
    global last_result
    queries = np.asarray(queries, dtype=np.float32)
    keys = np.asarray(keys, dtype=np.float32)
    v = np.asarray(v, dtype=np.float32)

    import ml_dtypes

    if "nc" not in _CACHE:
        _CACHE["nc"] = _build_nc()
        _CACHE["tables"] = _svd_tables()
    nc = _CACHE["nc"]
    grid, A, Bt = _CACHE["tables"]

    F8 = ml_dtypes.float8_e4m3
    sv = np.sqrt(np.abs(v))
    gs = (np.sign(v) * sv).astype(np.float64)

    in_maps = []
    for c in range(NCORES):
        b, half = c // 2, c % 2
        qs = queries[b, half * MPC: (half + 1) * MPC, :].astype(np.float64)
        ks = keys[b].astype(np.float64)
        gf = [np.interp(qs, grid, A[:, r]).T * gs[:, None] for r in range(R)]
        kf = [np.interp(ks, grid, Bt[:, r]).T * sv[:, None] for r in range(R)]

        feat8 = np.empty((H, FCOLS), dtype=F8)

        def put_g(base, g0, g1):
            for t in range(4):
                feat8[:, base + t * 256: base + t * 256 + 128] = g0[
                    :, t * 128: (t + 1) * 128
                ].astype(F8)
                feat8[:, base + t * 256 + 128: base + (t + 1) * 256] = g1[
                    :, t * 128: (t + 1) * 128
                ].astype(F8)

        def put_k(base, k0, k1):
            for hh in range(2):
                cols = slice(hh * 512, (hh + 1) * 512)
                feat8[:, base + hh * 1024: base + hh * 1024 + 512] = k0[
                    :, cols
                ].astype(F8)
                feat8[:, base + hh * 1024 + 512: base + (hh + 1) * 1024] = k1[
                    :, cols
                ].astype(F8)

        gh = [gf[r].astype(F8).astype(np.float64) for r in range(2)]
        kh = [kf[r].astype(F8).astype(np.float64) for r in range(2)]
        put_g(SECA, gh[0], gh[1])
        put_k(SECB, kh[0], kh[1])
        put_g(SECC, gf[0] - gh[0], gf[1] - gh[1])
        put_k(SECD, kf[0] - kh[0], kf[1] - kh[1])
        for p in range(NPAIR):
            r0, r1 = 2 + 2 * p, 3 + 2 * p
            put_g(SECE + p * 3072, gf[r0], gf[r1])
            put_k(SECE + p * 3072 + 1024, kf[r0], kf[r1])
        in_maps.append({"feat8": np.ascontiguousarray(feat8)})

    trace = bool(os.environ.get("KERNEL_TRACE"))
    res = run_bass_kernel_spmd(
        nc, in_maps, core_ids=list(range(NCORES)), trace=trace
    )
    last_result = res

    full = np.empty((B, M, N), np.float32)
    for c in range(NCORES):
        b, half = c // 2, c % 2
        full[b, half * MPC: (half + 1) * MPC, :] = res.results[c]["out"].astype(
            np.float32
        )
    return full
